# revision 18
# baseline (speedup 1.0000x reference)
"""GCN encoder (2-layer GCNConv, PyG-style) on 8 Trainium2 NeuronCores.

Sharding: nodes row-sharded 6250/core; edges partitioned by destination-node
owner; per-core segment-sum over 128-dst-slot windows via selection-matrix
matmuls.

v2 pipeline layout (vs v1):
  - table1 is stored rank-ROTATED per core (own rank first), so the replicated
    x @ W1 GEMM doubles as the own-rows pass (own1 rows copied straight out of
    the same PSUM tiles).
  - table1 split in two HALVES (ranks 0-3 / 4-7 in rotated order).  L1 gathers
    for half A start as soon as the A half of the GEMM is written (~1/2 into
    the GEMM) instead of after the whole table.
  - layer-2 table is all-gathered in FOUR window-range PIECES, each triggered
    as soon as L1 finishes that range of windows on all cores.  L2 aggregation
    runs piece-major, accumulating window partials in SBUF f32; the last piece
    writes output.  This removes the long serial AG tail of v1.
  - slot columns stored bf16 so the IS_EQ selection-matrix build runs at the
    16-bit DVE rate.

norm = dinv[src]*dinv[dst] is folded into table scaling:
  table1 = dinv .* (x @ W1)
  g~     = dinv^2 .* relu(segsum1)
  table2 = g~ @ W2
  out    = dinv .* segsum2
which is exact for b1 == 0 (the reference uses zero biases).

Self-loop messages never go through the gather path: their contribution to a
window's segment-sum is the core's own table rows, added with one identity
matmul per window from an SBUF-resident copy of the table shard.
"""

import os
import numpy as np
import ml_dtypes

import concourse.bacc as bacc
import concourse.tile as tile
from concourse import bass, mybir
from concourse.bass_utils import run_bass_kernel_spmd
from concourse.library_config import mlp

N = 50000
INC, HID, OUTC = 256, 256, 128
NCORES = 8
RPC = N // NCORES            # 6250 rows per core
WPC = (RPC + 127) // 128     # 49 windows per core
RPAD = WPC * 128             # 6272
NROWS = NCORES * RPAD        # 50176 table rows
HROWS = NROWS // 2           # 25088 rows per half (int16-indexable)

GRP1 = 2                     # windows per L1 gather group
NG1 = (WPC + GRP1 - 1) // GRP1   # 25
GRP2 = 4                     # windows per L2 gather group
NG2 = (WPC + GRP2 - 1) // GRP2   # 13

# AG pieces: window ranges (start, count); L1 GRP1 groups and L2 GRP2 groups
# both align with these boundaries (12 = 6*GRP1 = 3*GRP2).
PIECES = [(0, 12), (12, 12), (24, 12), (36, 13)]
NP = len(PIECES)

CH = 14                      # row-tiles per GEMM chunk (196/14 = 14 per half)


def _win_piece(w):
    for p, (w0, wn) in enumerate(PIECES):
        if w0 <= w < w0 + wn:
            return p
    raise AssertionError(w)


def _wrap_idx16(idx_seq, TT):
    """[TT*128] int32 -> [128, TT*8] int16 wrapped/replicated gather layout."""
    a = idx_seq.astype(np.int16).reshape(-1, 16).T      # [16, TT*8]
    return np.tile(a, (8, 1))


def _preprocess(edge_index):
    """Edge partitioning / ordering and normalization constants (host)."""
    src = np.asarray(edge_index[0], np.int64)
    dst = np.asarray(edge_index[1], np.int64)

    # degrees include the self-loops the reference adds
    deg = (np.bincount(dst, minlength=N) + 1).astype(np.float64)
    dinv = (1.0 / np.sqrt(deg)).astype(np.float32)

    owner = dst // RPC
    dstl = dst - owner * RPC
    win = dstl >> 7
    slot = dstl & 127
    src_rank = src // RPC
    srl = src - src_rank * RPC

    # ---- L1 streams: key = (owner, window, half) in ROTATED rank order
    # rotated rank for core c: (src_rank - c) % 8 ; half = rot // 4
    # rotated-half-local row: (rot % 4) * RPAD + srl
    # ---- L2 streams: key = (owner, window, piece) in canonical piece layout
    src_win = srl >> 7
    p2 = np.minimum(src_win // 12, 3)
    p2_w0 = np.array([w0 for (w0, _) in PIECES], np.int64)
    p2_wn = np.array([wn for (_, wn) in PIECES], np.int64)
    row2 = src_rank * (p2_wn[p2] * 128) + (srl - p2_w0[p2] * 128)

    # per-core counts for L1 (rotation differs per core)
    cnt1 = np.zeros((NCORES, WPC, 2), np.int64)
    cnt2 = np.zeros((NCORES, WPC, NP), np.int64)
    for c in range(NCORES):
        m = owner == c
        rot = (src_rank[m] - c) % NCORES
        h = rot // 4
        np.add.at(cnt1[c], (win[m], h), 1)
        np.add.at(cnt2[c], (win[m], p2[m]), 1)

    Twh1 = (cnt1.max(axis=0) + 127) // 128   # [WPC, 2]
    Twh2 = (cnt2.max(axis=0) + 127) // 128   # [WPC, NP]

    # ---- L1 unit emission order: full A-half pass (into partials), then
    # B-half pass (closing each window).  The A pass starts as soon as the
    # first half of the table GEMM is written.
    units1 = [(g, 0) for g in range(NG1)] + [(g, 1) for g in range(NG1)]

    base1 = np.zeros((WPC, 2), np.int64)
    pos = 0
    for (g, h) in units1:
        for w in range(g * GRP1, min((g + 1) * GRP1, WPC)):
            base1[w, h] = pos
            pos += Twh1[w, h]
    TT1 = pos

    # ---- L2 unit order: piece-major
    units2 = [(g, p) for p in range(NP) for g in range(NG2)]
    base2 = np.zeros((WPC, NP), np.int64)
    pos = 0
    for (g, p) in units2:
        for w in range(g * GRP2, min((g + 1) * GRP2, WPC)):
            base2[w, p] = pos
            pos += Twh2[w, p]
    TT2 = pos

    idx1 = np.empty((NCORES, 128, TT1 * 8), np.int16)
    slots1 = np.empty((NCORES, 128, TT1), np.float32)
    idx2 = np.empty((NCORES, 128, TT2 * 8), np.int16)
    slots2 = np.empty((NCORES, 128, TT2), np.float32)

    for c in range(NCORES):
        m = owner == c
        w_c = win[m]
        slot_c = slot[m]
        srl_c = srl[m]
        rot_c = (src_rank[m] - c) % NCORES
        h_c = rot_c // 4
        row1_c = (rot_c % 4) * RPAD + srl_c
        p2_c = p2[m]
        row2_c = row2[m]

        # L1: bucket sort by (w, h)
        key = w_c * 2 + h_c
        order = np.argsort(key, kind="stable")
        iseq = np.zeros(TT1 * 128, np.int32)
        sseq = np.full(TT1 * 128, 128, np.int32)
        counts = np.bincount(key, minlength=WPC * 2).reshape(WPC, 2)
        starts = np.concatenate([[0], np.cumsum(counts.reshape(-1))])
        r1s = row1_c[order]
        sls = slot_c[order]
        for w in range(WPC):
            for h in range(2):
                n = counts[w, h]
                if n == 0:
                    continue
                s0 = starts[w * 2 + h]
                p0 = base1[w, h] * 128
                iseq[p0 : p0 + n] = r1s[s0 : s0 + n]
                sseq[p0 : p0 + n] = sls[s0 : s0 + n]
        idx1[c] = _wrap_idx16(iseq, TT1)
        slots1[c] = sseq.astype(np.float32).reshape(TT1, 128).T

        # L2: bucket sort by (w, piece)
        key = w_c * NP + p2_c
        order = np.argsort(key, kind="stable")
        iseq = np.zeros(TT2 * 128, np.int32)
        sseq = np.full(TT2 * 128, 128, np.int32)
        counts = np.bincount(key, minlength=WPC * NP).reshape(WPC, NP)
        starts = np.concatenate([[0], np.cumsum(counts.reshape(-1))])
        r2s = row2_c[order]
        sls = slot_c[order]
        for w in range(WPC):
            for p in range(NP):
                n = counts[w, p]
                if n == 0:
                    continue
                s0 = starts[w * NP + p]
                p0 = base2[w, p] * 128
                iseq[p0 : p0 + n] = r2s[s0 : s0 + n]
                sseq[p0 : p0 + n] = sls[s0 : s0 + n]
        idx2[c] = _wrap_idx16(iseq, TT2)
        slots2[c] = sseq.astype(np.float32).reshape(TT2, 128).T

    # per-core per-window dinv columns for own rows
    dcol1 = np.zeros((NCORES, 128, WPC), np.float32)
    for c in range(NCORES):
        d = np.zeros(RPAD, np.float32)
        d[:RPC] = dinv[c * RPC : (c + 1) * RPC]
        dcol1[c] = d.reshape(WPC, 128).T
    dcol2 = dcol1 * dcol1

    return (idx1, slots1, Twh1, base1, TT1, units1,
            idx2, slots2, Twh2, base2, TT2, units2, dcol1, dcol2, dinv)


def _build(Twh1, base1, TT1, units1, Twh2, base2, TT2, units2):
    nc = bacc.Bacc("TRN2", num_devices=NCORES, num_swdge_queues=4)
    f32 = mybir.dt.float32
    bf = mybir.dt.bfloat16

    xt_d = nc.dram_tensor("xtf", [2, 128, NROWS], bf, kind="ExternalInput")
    w1_d = nc.dram_tensor("w1", [2, 128, HID], bf, kind="ExternalInput")
    w2_d = nc.dram_tensor("w2", [2, 128, OUTC], bf, kind="ExternalInput")
    iota_d = nc.dram_tensor("iota", [128, 128], bf, kind="ExternalInput")
    ident_d = nc.dram_tensor("ident", [128, 128], bf, kind="ExternalInput")
    dc1_d = nc.dram_tensor("dcol1", [128, WPC], f32, kind="ExternalInput")
    dc2_d = nc.dram_tensor("dcol2", [128, WPC], f32, kind="ExternalInput")
    idx1_d = nc.dram_tensor("idx1", [128, TT1 * 8], mybir.dt.int16, kind="ExternalInput")
    sl1_d = nc.dram_tensor("slots1", [128, TT1], bf, kind="ExternalInput")
    idx2_d = nc.dram_tensor("idx2", [128, TT2 * 8], mybir.dt.int16, kind="ExternalInput")
    sl2_d = nc.dram_tensor("slots2", [128, TT2], bf, kind="ExternalInput")
    out_d = nc.dram_tensor("out", [RPAD, OUTC], f32, kind="ExternalOutput")

    # tiles per L1 (group, half) unit and L2 (group, piece) unit
    Tg1 = np.zeros((NG1, 2), np.int64)
    for g in range(NG1):
        for h in range(2):
            Tg1[g, h] = sum(int(Twh1[w, h])
                            for w in range(g * GRP1, min((g + 1) * GRP1, WPC)))
    Tg2 = np.zeros((NG2, NP), np.int64)
    for g in range(NG2):
        for p in range(NP):
            Tg2[g, p] = sum(int(Twh2[w, p])
                            for w in range(g * GRP2, min((g + 1) * GRP2, WPC)))

    with tile.TileContext(nc) as tc:
        nc.gpsimd.load_library(mlp)
        with (
            tc.tile_pool(name="const", bufs=1) as cpool,
            tc.tile_pool(name="gt", bufs=1) as gtpool,
            tc.tile_pool(name="xts", bufs=2) as xtpool,
            tc.tile_pool(name="evac", bufs=2) as epool,
            tc.tile_pool(name="small", bufs=3) as smpool,
            tc.tile_pool(name="msg1", bufs=4) as m1pool,
            tc.tile_pool(name="msg2", bufs=3) as m2pool,
            tc.tile_pool(name="sel", bufs=3) as spool,
            tc.tile_pool(name="p256", bufs=3, space="PSUM") as p256,
            tc.tile_pool(name="p128", bufs=2, space="PSUM") as p128,
            tc.tile_pool(name="ptr", bufs=1, space="PSUM") as ptr,
            tc.tile_pool(name="pl2", bufs=2, space="PSUM") as pl2,
            tc.tile_pool(name="dram", bufs=1, space="DRAM") as dram,
        ):
            # ---- constants to SBUF
            w1_s = cpool.tile([128, 2, HID], bf)
            w2_s = cpool.tile([128, 2, OUTC], bf)
            iota_s = cpool.tile([128, 128], bf)
            ident_s = cpool.tile([128, 128], bf)
            dc1_s = cpool.tile([128, WPC], f32)
            dc2_s = cpool.tile([128, WPC], f32)
            idx1_s = cpool.tile([128, TT1 * 8], mybir.dt.int16)
            sl1_s = cpool.tile([128, TT1], bf)
            idx2_s = cpool.tile([128, TT2 * 8], mybir.dt.int16)
            sl2_s = cpool.tile([128, TT2], bf)
            own1_s = gtpool.tile([128, WPC, HID], bf)   # own table1 rows
            own2_s = gtpool.tile([128, WPC, OUTC], bf)  # own table2 rows
            parta_s = gtpool.tile([128, WPC, HID], bf)  # L1 A-half partials
            part_s = gtpool.tile([128, WPC, OUTC], bf)  # L2 window partials
            for k in range(2):
                nc.sync.dma_start(w1_s[:, k, :], w1_d[k])
                nc.sync.dma_start(w2_s[:, k, :], w2_d[k])
            nc.sync.dma_start(iota_s[:], iota_d[:])
            nc.sync.dma_start(ident_s[:], ident_d[:])
            nc.sync.dma_start(dc1_s[:], dc1_d[:])
            nc.sync.dma_start(dc2_s[:], dc2_d[:])
            nc.sync.dma_start(idx1_s[:], idx1_d[:])
            nc.sync.dma_start(sl1_s[:], sl1_d[:])
            nc.sync.dma_start(idx2_s[:], idx2_d[:])
            nc.sync.dma_start(sl2_s[:], sl2_d[:])

            tb1a = dram.tile([HROWS, HID], bf)
            tb1b = dram.tile([HROWS, HID], bf)
            ag_in = [dram.tile([wn * 128, OUTC], bf, name=f"ag_in{p}")
                     for p, (_, wn) in enumerate(PIECES)]
            tb2p = [dram.tile([NCORES * wn * 128, OUTC], bf, name=f"tb2p{p}")
                    for p, (_, wn) in enumerate(PIECES)]

            # ---- phase 1: replicated table1 = (dinv .* x) @ W1, rotated
            # rank-major (own rank first).  Rows < RPAD also feed own1_s.
            with nc.named_scope("p1"):
                TBLT = NROWS // 128          # 392 row tiles
                for c0 in range(0, TBLT, CH):
                    xt_t = xtpool.tile([128, 2, CH * 128], bf, tag="xt")
                    nc.scalar.dma_start(
                        xt_t[:],
                        xt_d[:, :, c0 * 128 : (c0 + CH) * 128].rearrange("k p n -> p k n"))
                    ev = epool.tile([128, CH, HID], bf, tag="xw")
                    for j in range(CH):
                        rt = c0 + j
                        pool_j = p256 if j % 2 == 0 else p128
                        ps = pool_j.tile([128, HID], f32,
                                         tag="p256" if j % 2 == 0 else "p128")
                        for k in range(2):
                            nc.tensor.matmul(
                                ps[:], lhsT=xt_t[:, k, j * 128 : (j + 1) * 128],
                                rhs=w1_s[:, k, :], start=(k == 0), stop=(k == 1))
                        # psum evacuations go through ACT: DVE 2-port ops lock
                        # GpSimd out of SBUF and stall gather descriptor gen
                        nc.scalar.activation(ev[:, j, :], ps[:],
                                             mybir.ActivationFunctionType.Copy)
                        if rt < WPC:  # own rows (rotated rank 0 comes first)
                            nc.scalar.activation(own1_s[:, rt, :], ps[:],
                                                 mybir.ActivationFunctionType.Copy)
                    tb, r0 = (tb1a, c0 * 128) if c0 < TBLT // 2 else (tb1b, c0 * 128 - HROWS)
                    nc.sync.dma_start(
                        tb[r0 : r0 + CH * 128, :].rearrange("(j p) c -> p j c", p=128),
                        ev[:])

            # ---- gather unit: gathers + S build for one (group, sub) stream
            qctr = [0]

            def gather_unit(b, T, tbl, width, idx_s, sl_s, mpool, mtag):
                m_s = mpool.tile([128, T, width], bf, tag=mtag)
                nc.gpsimd.dma_gather(
                    m_s[:], tbl[:, :], idx_s[:, b * 8 : (b + T) * 8],
                    T * 128, T * 128, width,
                    single_packet=False, queue_num=qctr[0] % 4)
                qctr[0] += 1
                S_s = spool.tile([128, T, 128], bf, tag="sel")
                nc.vector.tensor_tensor(
                    out=S_s[:],
                    in0=sl_s[:, b : b + T, None].to_broadcast([128, T, 128]),
                    in1=iota_s[:, None, :].to_broadcast([128, T, 128]),
                    op=mybir.AluOpType.is_equal)
                return m_s, S_s

            def win_mms(ps, m_s, S_s, t0, n, first, last):
                for t in range(n):
                    nc.tensor.matmul(ps[:], lhsT=S_s[:, t0 + t, :],
                                     rhs=m_s[:, t0 + t, :],
                                     start=(first and t == 0),
                                     stop=(last and t == n - 1))

            # ---- phase 2: L1 aggregation in two half-passes.
            # Pass A (as soon as tb1a is written): per-window A-half segment
            # sums, parked in bf16 partials.  Pass B: B-half sums + A partial
            # + self-loop close each window, producing table2 rows; AG piece p
            # fires when its windows finish.
            has_a = [False] * WPC

            def l1_unit_a(g):
                ws = list(range(g * GRP1, min((g + 1) * GRP1, WPC)))
                T = int(Tg1[g, 0])
                if T == 0:
                    return
                b = int(base1[ws[0], 0])
                m_s, S_s = gather_unit(b, T, tb1a, HID, idx1_s, sl1_s,
                                       m1pool, "msg1")
                for w in ws:
                    n = int(Twh1[w, 0])
                    if n == 0:
                        continue
                    ps = p256.tile([128, HID], f32, tag="p256")
                    win_mms(ps, m_s, S_s, int(base1[w, 0]) - b, n, True, True)
                    nc.scalar.activation(parta_s[:, w, :], ps[:],
                                         mybir.ActivationFunctionType.Copy)
                    has_a[w] = True

            def l1_unit_b(g):
                ws = list(range(g * GRP1, min((g + 1) * GRP1, WPC)))
                T = int(Tg1[g, 1])
                if T > 0:
                    b = int(base1[ws[0], 1])
                    m_s, S_s = gather_unit(b, T, tb1b, HID, idx1_s, sl1_s,
                                           m1pool, "msg1")
                for w in ws:
                    ps = p256.tile([128, HID], f32, tag="p256")
                    started = False
                    n = int(Twh1[w, 1])
                    if T > 0 and n > 0:
                        win_mms(ps, m_s, S_s, int(base1[w, 1]) - b, n, True, False)
                        started = True
                    if has_a[w]:
                        nc.tensor.matmul(ps[:], lhsT=ident_s[:],
                                         rhs=parta_s[:, w, :],
                                         start=not started, stop=False)
                        started = True
                    # self-loop contribution closes the accumulation
                    nc.tensor.matmul(ps[:], lhsT=ident_s[:],
                                     rhs=own1_s[:, w, :],
                                     start=not started, stop=True)
                    g_s = smpool.tile([128, HID], bf, tag="g")
                    nc.scalar.activation(g_s[:], ps[:],
                                         mybir.ActivationFunctionType.Relu,
                                         scale=dc2_s[:, w : w + 1])
                    gtw = smpool.tile([128, 2, 128], bf, tag="gtw")
                    for k in range(2):
                        pt = ptr.tile([128, 128], bf, tag="pt")
                        nc.tensor.transpose(pt[:], g_s[:, k * 128 : (k + 1) * 128],
                                            ident_s[:])
                        nc.scalar.activation(gtw[:, k, :], pt[:],
                                             mybir.ActivationFunctionType.Copy)
                    ps2 = p128.tile([128, OUTC], f32, tag="p128")
                    for k in range(2):
                        nc.tensor.matmul(ps2[:],
                                         lhsT=gtw[:, k, :],
                                         rhs=w2_s[:, k, :],
                                         start=(k == 0), stop=(k == 1))
                    nc.scalar.activation(own2_s[:, w, :], ps2[:],
                                         mybir.ActivationFunctionType.Copy)
                    p = _win_piece(w)
                    w0 = PIECES[p][0]
                    nc.sync.dma_start(
                        ag_in[p][(w - w0) * 128 : (w - w0 + 1) * 128, :],
                        own2_s[:, w, :])

            with nc.named_scope("l1a"):
                for g in range(NG1):
                    l1_unit_a(g)
            with nc.named_scope("l1b"):
                for g in range(NG1):
                    l1_unit_b(g)
                    # fire AG piece p once its last window was evacuated
                    w_last = min((g + 1) * GRP1, WPC) - 1
                    for p, (w0, wn) in enumerate(PIECES):
                        if w_last == w0 + wn - 1:
                            with nc.named_scope(f"ag{p}"):
                                nc.gpsimd.collective_compute(
                                    "AllGather", mybir.AluOpType.bypass,
                                    replica_groups=[list(range(NCORES))],
                                    ins=[ag_in[p].opt()], outs=[tb2p[p].opt()])

            # ---- phase 3: L2 aggregation piece-major.  Each piece pass folds
            # the running bf16 partial back in through an identity matmul, so
            # windows with no messages in a piece are untouched.  The last
            # piece scales by dinv and writes output.
            for p in range(NP):
                with nc.named_scope(f"l2p{p}"):
                    for g in range(NG2):
                        ws = list(range(g * GRP2, min((g + 1) * GRP2, WPC)))
                        T = int(Tg2[g, p])
                        if T > 0:
                            b = int(base2[ws[0], p])
                            m_s, S_s = gather_unit(b, T, tb2p[p], OUTC,
                                                   idx2_s, sl2_s, m2pool, "msg2")
                        for w in ws:
                            n = int(Twh2[w, p])
                            has_mm = T > 0 and n > 0
                            if not has_mm and 0 < p < NP - 1:
                                continue  # running partial unchanged
                            ps = pl2.tile([128, OUTC], f32, tag="pl2")
                            started = False
                            if has_mm:
                                t0 = int(base2[w, p]) - b
                                win_mms(ps, m_s, S_s, t0, n, True, False)
                                started = True
                            # fold in running partial (piece 0: the self-loop)
                            prev = own2_s if p == 0 else part_s
                            nc.tensor.matmul(ps[:], lhsT=ident_s[:],
                                             rhs=prev[:, w, :],
                                             start=not started, stop=True)
                            if p < NP - 1:
                                nc.scalar.activation(
                                    part_s[:, w, :], ps[:],
                                    mybir.ActivationFunctionType.Copy)
                            else:
                                o_s = smpool.tile([128, OUTC], f32, tag="o")
                                nc.scalar.activation(
                                    o_s[:], ps[:],
                                    mybir.ActivationFunctionType.Copy,
                                    scale=dc1_s[:, w : w + 1])
                                nc.sync.dma_start(out_d[w * 128 : (w + 1) * 128, :],
                                                  o_s[:])

    nc.compile()
    return nc


def kernel(x, edge_index, W1, b1, W2, b2):
    x = np.asarray(x, np.float32)
    W1 = np.asarray(W1, np.float32)
    W2 = np.asarray(W2, np.float32)
    assert not np.any(np.asarray(b1)) and not np.any(np.asarray(b2)), \
        "kernel assumes zero biases (as in the reference setup)"

    (idx1, slots1, Twh1, base1, TT1, units1,
     idx2, slots2, Twh2, base2, TT2, units2,
     dcol1, dcol2, dinv) = _preprocess(np.asarray(edge_index))
    nc = _build(Twh1, base1, TT1, units1, Twh2, base2, TT2, units2)

    iota = np.broadcast_to(np.arange(128, dtype=np.float32), (128, 128)).astype(ml_dtypes.bfloat16)
    ident = np.eye(128, dtype=np.float32).astype(ml_dtypes.bfloat16)
    w1_in = np.ascontiguousarray(W1.reshape(2, 128, HID)).astype(ml_dtypes.bfloat16)
    w2_in = np.ascontiguousarray(W2.reshape(2, 128, OUTC)).astype(ml_dtypes.bfloat16)

    # canonical transposed scaled x: [2, 128, rank, RPAD]
    xd = (x * dinv[:, None]).astype(np.float32)
    xtc = np.zeros((256, NCORES, RPAD), np.float32)
    for rho in range(NCORES):
        xtc[:, rho, :RPC] = xd[rho * RPC : (rho + 1) * RPC].T
    xtc = xtc.reshape(2, 128, NCORES, RPAD).astype(ml_dtypes.bfloat16)

    in_maps = []
    for c in range(NCORES):
        rolled = np.concatenate([xtc[:, :, c:, :], xtc[:, :, :c, :]], axis=2)
        in_maps.append({
            "xtf": np.ascontiguousarray(rolled.reshape(2, 128, NROWS)),
            "w1": w1_in, "w2": w2_in, "iota": iota, "ident": ident,
            "dcol1": dcol1[c], "dcol2": dcol2[c],
            "idx1": idx1[c], "slots1": slots1[c].astype(ml_dtypes.bfloat16),
            "idx2": idx2[c], "slots2": slots2[c].astype(ml_dtypes.bfloat16),
        })

    trace = bool(int(os.environ.get("GCN_KERNEL_TRACE", "0")))
    try:
        res = run_bass_kernel_spmd(nc, in_maps, core_ids=list(range(NCORES)), trace=trace)
    except Exception:
        # rare transient NRT exec failure: retry once on a fresh dispatch
        time_mod = __import__("time"); time_mod.sleep(2.0)
        res = run_bass_kernel_spmd(nc, in_maps, core_ids=list(range(NCORES)), trace=False)
    kernel.last_results = res
    if trace:
        print(f"HW exec time: {res.exec_time_ns} ns")
        kernel.last_exec_time_ns = res.exec_time_ns

    out = np.concatenate([res.results[c]["out"][:RPC] for c in range(NCORES)], axis=0)
    return out.astype(np.float32)


# revision 19
# speedup vs baseline: 1.1765x; 1.1765x over previous
"""GCN encoder (2-layer GCNConv, PyG-style) on 8 Trainium2 NeuronCores.

Sharding: nodes row-sharded 6250/core; edges partitioned by destination-node
owner; per-core segment-sum over 128-dst-slot windows via selection-matrix
matmuls.

v2.2 design:
  - table1 stored rank-ROTATED per core (own rank first): the replicated
    x @ W1 GEMM doubles as the own-rows pass.  Table split in two halves
    (rotated ranks 0-3 / 4-7) for int16 gather indices.
  - L1 processed in paired units per 2-window group: gather(A) + gather(B)
    + full window close (relu -> transpose -> @W2 -> table2 row).
  - selection matrices are NOT built on device: the host ships them as fp8
    tiles (0/1 exact) streamed in with bulk DMA.  This keeps the DVE almost
    idle — DVE 2-port ops lock GpSimd out of SBUF and stall the gather
    descriptor generation, which paces the whole kernel.
  - layer-2 table all-gathered in FOUR window-range PIECES, each fired as
    soon as L1 finishes that window range; L2 aggregation runs piece-major,
    folding a bf16 running partial through identity matmuls.

norm = dinv[src]*dinv[dst] folded into table scaling:
  table1 = dinv .* (x @ W1);  g~ = dinv^2 .* relu(segsum1)
  table2 = g~ @ W2;           out = dinv .* segsum2       (biases are zero)

Self-loop messages bypass the gather path (one identity matmul per window
from SBUF-resident own-row tables).
"""

import os
import numpy as np
import ml_dtypes

import concourse.bacc as bacc
import concourse.tile as tile
from concourse import bass, mybir
from concourse.bass_utils import run_bass_kernel_spmd
from concourse.library_config import mlp

N = 50000
INC, HID, OUTC = 256, 256, 128
NCORES = 8
RPC = N // NCORES            # 6250 rows per core
WPC = (RPC + 127) // 128     # 49 windows per core
RPAD = WPC * 128             # 6272
NROWS = NCORES * RPAD        # 50176 table rows
HROWS = NROWS // 2           # 25088 rows per half (int16-indexable)

GRP1 = 2                     # windows per L1 group
NG1 = (WPC + GRP1 - 1) // GRP1   # 25
GRP2 = 4                     # windows per L2 gather group
NG2 = (WPC + GRP2 - 1) // GRP2   # 13

# AG pieces: window ranges (start, count); L1/L2 group sizes divide 12.
PIECES = [(0, 12), (12, 12), (24, 12), (36, 13)]
NP = len(PIECES)

CH = 14                      # row-tiles per GEMM chunk (196/14 = 14 per half)


def _win_piece(w):
    for p, (w0, wn) in enumerate(PIECES):
        if w0 <= w < w0 + wn:
            return p
    raise AssertionError(w)


def _wrap_idx16(idx_seq):
    """[TT*128] int32 -> [128, TT*8] int16 wrapped/replicated gather layout."""
    a = idx_seq.astype(np.int16).reshape(-1, 16).T      # [16, TT*8]
    return np.tile(a, (8, 1))


def _preprocess(edge_index):
    """Edge partitioning / ordering and normalization constants (host)."""
    src = np.asarray(edge_index[0], np.int64)
    dst = np.asarray(edge_index[1], np.int64)

    # degrees include the self-loops the reference adds
    deg = (np.bincount(dst, minlength=N) + 1).astype(np.float64)
    dinv = (1.0 / np.sqrt(deg)).astype(np.float32)

    owner = dst // RPC
    dstl = dst - owner * RPC
    win = dstl >> 7
    slot = dstl & 127
    src_rank = src // RPC
    srl = src - src_rank * RPC

    # L2 piece of each source node (canonical layout)
    src_win = srl >> 7
    p2 = np.minimum(src_win // 12, 3)
    p2_w0 = np.array([w0 for (w0, _) in PIECES], np.int64)
    p2_wn = np.array([wn for (_, wn) in PIECES], np.int64)
    row2 = src_rank * (p2_wn[p2] * 128) + (srl - p2_w0[p2] * 128)

    # per-core counts (L1 rotation differs per core)
    cnt1 = np.zeros((NCORES, WPC, 2), np.int64)
    cnt2 = np.zeros((NCORES, WPC, NP), np.int64)
    for c in range(NCORES):
        m = owner == c
        rot = (src_rank[m] - c) % NCORES
        np.add.at(cnt1[c], (win[m], rot // 4), 1)
        np.add.at(cnt2[c], (win[m], p2[m]), 1)

    Twh1 = (cnt1.max(axis=0) + 127) // 128   # [WPC, 2]
    Twh2 = (cnt2.max(axis=0) + 127) // 128   # [WPC, NP]

    # stream layouts: L1 group-major, sub inside group; L2 piece-major
    base1 = np.zeros((WPC, 2), np.int64)
    pos = 0
    for g in range(NG1):
        for h in range(2):
            for w in range(g * GRP1, min((g + 1) * GRP1, WPC)):
                base1[w, h] = pos
                pos += Twh1[w, h]
    TT1 = pos

    base2 = np.zeros((WPC, NP), np.int64)
    pos = 0
    for p in range(NP):
        for g in range(NG2):
            for w in range(g * GRP2, min((g + 1) * GRP2, WPC)):
                base2[w, p] = pos
                pos += Twh2[w, p]
    TT2 = pos

    idx1 = np.empty((NCORES, 128, TT1 * 8), np.int16)
    slots1 = np.empty((NCORES, TT1 * 128), np.int32)
    idx2 = np.empty((NCORES, 128, TT2 * 8), np.int16)
    slots2 = np.empty((NCORES, TT2 * 128), np.int32)

    for c in range(NCORES):
        m = owner == c
        w_c = win[m]
        slot_c = slot[m]
        srl_c = srl[m]
        rot_c = (src_rank[m] - c) % NCORES
        h_c = rot_c // 4
        row1_c = (rot_c % 4) * RPAD + srl_c

        def fill(key, nbuck, basef, rows, TT):
            order = np.argsort(key, kind="stable")
            iseq = np.zeros(TT * 128, np.int32)
            sseq = np.full(TT * 128, 128, np.int32)
            counts = np.bincount(key, minlength=nbuck)
            starts = np.concatenate([[0], np.cumsum(counts)])
            rs = rows[order]
            sls = slot_c[order]
            for k in range(nbuck):
                n = counts[k]
                if n == 0:
                    continue
                s0 = starts[k]
                p0 = basef(k) * 128
                iseq[p0 : p0 + n] = rs[s0 : s0 + n]
                sseq[p0 : p0 + n] = sls[s0 : s0 + n]
            return iseq, sseq

        iseq, sseq = fill(w_c * 2 + h_c, WPC * 2,
                          lambda k: base1[k // 2, k % 2], row1_c, TT1)
        idx1[c] = _wrap_idx16(iseq)
        slots1[c] = sseq

        iseq, sseq = fill(w_c * NP + p2[m], WPC * NP,
                          lambda k: base2[k // NP, k % NP], row2[m], TT2)
        idx2[c] = _wrap_idx16(iseq)
        slots2[c] = sseq

    # per-core per-window dinv columns for own rows
    dcol1 = np.zeros((NCORES, 128, WPC), np.float32)
    for c in range(NCORES):
        d = np.zeros(RPAD, np.float32)
        d[:RPC] = dinv[c * RPC : (c + 1) * RPC]
        dcol1[c] = d.reshape(WPC, 128).T
    dcol2 = dcol1 * dcol1

    return (idx1, slots1, Twh1, base1, TT1,
            idx2, slots2, Twh2, base2, TT2, dcol1, dcol2, dinv)


def _sel_tiles(slots, TT):
    """[TT*128] slot ids -> [128, TT, 128] fp8 selection tiles (S[p,t,j])."""
    s = slots.reshape(TT, 128).T                       # [128 part, TT]
    S = (s[:, :, None] == np.arange(128)[None, None, :])
    return S.astype(ml_dtypes.float8_e4m3fn)


def _build(Twh1, base1, TT1, Twh2, base2, TT2):
    nc = bacc.Bacc("TRN2", num_devices=NCORES, num_swdge_queues=4)
    f32 = mybir.dt.float32
    bf = mybir.dt.bfloat16
    f8 = mybir.dt.float8e4

    xt_d = nc.dram_tensor("xtf", [2, 128, NROWS], bf, kind="ExternalInput")
    w1_d = nc.dram_tensor("w1", [2, 128, HID], bf, kind="ExternalInput")
    w2_d = nc.dram_tensor("w2", [2, 128, OUTC], bf, kind="ExternalInput")
    ident_d = nc.dram_tensor("ident", [128, 128], bf, kind="ExternalInput")
    dc1_d = nc.dram_tensor("dcol1", [128, WPC], f32, kind="ExternalInput")
    dc2_d = nc.dram_tensor("dcol2", [128, WPC], f32, kind="ExternalInput")
    idx1_d = nc.dram_tensor("idx1", [128, TT1 * 8], mybir.dt.int16, kind="ExternalInput")
    idx2_d = nc.dram_tensor("idx2", [128, TT2 * 8], mybir.dt.int16, kind="ExternalInput")
    s1_d = nc.dram_tensor("sel1", [128, TT1, 128], f8, kind="ExternalInput")
    s2_d = nc.dram_tensor("sel2", [128, TT2, 128], f8, kind="ExternalInput")
    out_d = nc.dram_tensor("out", [RPAD, OUTC], f32, kind="ExternalOutput")

    Tg1 = np.zeros((NG1, 2), np.int64)
    for g in range(NG1):
        for h in range(2):
            Tg1[g, h] = sum(int(Twh1[w, h])
                            for w in range(g * GRP1, min((g + 1) * GRP1, WPC)))
    Tg2 = np.zeros((NG2, NP), np.int64)
    for g in range(NG2):
        for p in range(NP):
            Tg2[g, p] = sum(int(Twh2[w, p])
                            for w in range(g * GRP2, min((g + 1) * GRP2, WPC)))

    with tile.TileContext(nc) as tc:
        nc.gpsimd.load_library(mlp)
        with (
            tc.tile_pool(name="const", bufs=1) as cpool,
            tc.tile_pool(name="gt", bufs=1) as gtpool,
            tc.tile_pool(name="xts", bufs=2) as xtpool,
            tc.tile_pool(name="evac", bufs=2) as epool,
            tc.tile_pool(name="small", bufs=3) as smpool,
            tc.tile_pool(name="msg1", bufs=4) as m1pool,
            tc.tile_pool(name="msg2", bufs=3) as m2pool,
            tc.tile_pool(name="sel", bufs=4) as spool,
            tc.tile_pool(name="pwin", bufs=4, space="PSUM") as pwin,
            tc.tile_pool(name="ps2", bufs=1, space="PSUM") as ps2pool,
            tc.tile_pool(name="ptr", bufs=1, space="PSUM") as ptr,
            tc.tile_pool(name="pl2", bufs=2, space="PSUM") as pl2,
            tc.tile_pool(name="dram", bufs=1, space="DRAM") as dram,
        ):
            # ---- constants to SBUF
            w1_s = cpool.tile([128, 2, HID], bf)
            w2_s = cpool.tile([128, 2, OUTC], bf)
            ident_s = cpool.tile([128, 128], bf)
            dc1_s = cpool.tile([128, WPC], f32)
            dc2_s = cpool.tile([128, WPC], f32)
            idx1_s = cpool.tile([128, TT1 * 8], mybir.dt.int16)
            idx2_s = cpool.tile([128, TT2 * 8], mybir.dt.int16)
            own1_s = gtpool.tile([128, WPC, HID], bf)   # own table1 rows
            own2_s = gtpool.tile([128, WPC, OUTC], bf)  # own table2 rows
            part_s = gtpool.tile([128, WPC, OUTC], bf)  # L2 window partials
            for k in range(2):
                nc.sync.dma_start(w1_s[:, k, :], w1_d[k])
                nc.sync.dma_start(w2_s[:, k, :], w2_d[k])
            nc.sync.dma_start(ident_s[:], ident_d[:])
            nc.sync.dma_start(dc1_s[:], dc1_d[:])
            nc.sync.dma_start(dc2_s[:], dc2_d[:])
            nc.sync.dma_start(idx1_s[:], idx1_d[:])
            nc.sync.dma_start(idx2_s[:], idx2_d[:])

            tb1a = dram.tile([HROWS, HID], bf)
            tb1b = dram.tile([HROWS, HID], bf)
            ag_in = [dram.tile([wn * 128, OUTC], bf, name=f"ag_in{p}")
                     for p, (_, wn) in enumerate(PIECES)]
            tb2p = [dram.tile([NCORES * wn * 128, OUTC], bf, name=f"tb2p{p}")
                    for p, (_, wn) in enumerate(PIECES)]

            # ---- phase 1: replicated table1 = (dinv .* x) @ W1, rotated
            # rank-major (own rank first).  Rows < RPAD also feed own1_s.
            with nc.named_scope("p1"):
                TBLT = NROWS // 128          # 392 row tiles
                for c0 in range(0, TBLT, CH):
                    xt_t = xtpool.tile([128, 2, CH * 128], bf, tag="xt")
                    nc.scalar.dma_start(
                        xt_t[:],
                        xt_d[:, :, c0 * 128 : (c0 + CH) * 128].rearrange("k p n -> p k n"))
                    ev = epool.tile([128, CH, HID], bf, tag="xw")
                    for j in range(CH):
                        rt = c0 + j
                        ps = pwin.tile([128, HID], f32, tag="win")
                        for k in range(2):
                            nc.tensor.matmul(
                                ps[:], lhsT=xt_t[:, k, j * 128 : (j + 1) * 128],
                                rhs=w1_s[:, k, :], start=(k == 0), stop=(k == 1))
                        if j % 2 == 0:
                            nc.vector.tensor_copy(ev[:, j, :], ps[:])
                        else:
                            nc.scalar.activation(ev[:, j, :], ps[:],
                                                 mybir.ActivationFunctionType.Copy)
                        if rt < WPC:  # own rows (rotated rank 0 comes first)
                            nc.scalar.activation(own1_s[:, rt, :], ps[:],
                                                 mybir.ActivationFunctionType.Copy)
                    tb, r0 = (tb1a, c0 * 128) if c0 < TBLT // 2 else (tb1b, c0 * 128 - HROWS)
                    nc.sync.dma_start(
                        tb[r0 : r0 + CH * 128, :].rearrange("(j p) c -> p j c", p=128),
                        ev[:])

            qctr = [0]

            def gather_unit(b, T, tbl, width, idx_s, s_d, mpool, mtag, stag):
                m_s = mpool.tile([128, T, width], bf, tag=mtag)
                nc.gpsimd.dma_gather(
                    m_s[:], tbl[:, :], idx_s[:, b * 8 : (b + T) * 8],
                    T * 128, T * 128, width,
                    single_packet=False, queue_num=qctr[0] % 4)
                qctr[0] += 1
                S_s = spool.tile([128, T, 128], f8, tag=stag)
                nc.sync.dma_start(S_s[:], s_d[:, b : b + T, :])
                return m_s, S_s

            def win_mms(ps, m_s, S_s, t0, n, first, last):
                for t in range(n):
                    nc.tensor.matmul(ps[:], lhsT=S_s[:, t0 + t, :],
                                     rhs=m_s[:, t0 + t, :],
                                     start=(first and t == 0),
                                     stop=(last and t == n - 1))

            # ---- phase 2: L1 aggregation, paired A/B units per group, full
            # window close; AG piece p fires when its windows are done.
            with nc.named_scope("l1"):
                for g in range(NG1):
                    ws = list(range(g * GRP1, min((g + 1) * GRP1, WPC)))
                    units = {}
                    for h in range(2):
                        T = int(Tg1[g, h])
                        units[h] = (gather_unit(int(base1[ws[0], h]), T,
                                                tb1a if h == 0 else tb1b, HID,
                                                idx1_s, s1_d, m1pool, "msg1",
                                                "sel1")
                                    if T > 0 else (None, None))
                    for w in ws:
                        ps = pwin.tile([128, HID], f32, tag="win")
                        started = False
                        for h in range(2):
                            m_s, S_s = units[h]
                            n = int(Twh1[w, h])
                            if m_s is None or n == 0:
                                continue
                            t0 = int(base1[w, h]) - int(base1[ws[0], h])
                            win_mms(ps, m_s, S_s, t0, n, not started, False)
                            started = True
                        nc.tensor.matmul(ps[:], lhsT=ident_s[:],
                                         rhs=own1_s[:, w, :],
                                         start=not started, stop=True)
                        g_s = smpool.tile([128, HID], bf, tag="g")
                        nc.scalar.activation(g_s[:], ps[:],
                                             mybir.ActivationFunctionType.Relu,
                                             scale=dc2_s[:, w : w + 1])
                        gtw = smpool.tile([128, 2, 128], bf, tag="gtw")
                        for k in range(2):
                            pt = ptr.tile([128, 128], bf, tag="pt")
                            nc.tensor.transpose(pt[:], g_s[:, k * 128 : (k + 1) * 128],
                                                ident_s[:])
                            nc.vector.tensor_copy(gtw[:, k, :], pt[:])
                        ps2 = ps2pool.tile([128, OUTC], f32, tag="ps2")
                        for k in range(2):
                            nc.tensor.matmul(ps2[:], lhsT=gtw[:, k, :],
                                             rhs=w2_s[:, k, :],
                                             start=(k == 0), stop=(k == 1))
                        nc.vector.tensor_copy(own2_s[:, w, :], ps2[:])
                        p = _win_piece(w)
                        w0 = PIECES[p][0]
                        nc.sync.dma_start(
                            ag_in[p][(w - w0) * 128 : (w - w0 + 1) * 128, :],
                            own2_s[:, w, :])
                    # fire AG piece p once its last window was evacuated
                    w_last = ws[-1]
                    for p, (w0, wn) in enumerate(PIECES):
                        if w_last == w0 + wn - 1:
                            with nc.named_scope(f"ag{p}"):
                                nc.gpsimd.collective_compute(
                                    "AllGather", mybir.AluOpType.bypass,
                                    replica_groups=[list(range(NCORES))],
                                    ins=[ag_in[p].opt()], outs=[tb2p[p].opt()])

            # ---- phase 3: L2 aggregation piece-major; bf16 running partial
            # folded back in via identity matmuls; last piece writes output.
            for p in range(NP):
                with nc.named_scope(f"l2p{p}"):
                    for g in range(NG2):
                        ws = list(range(g * GRP2, min((g + 1) * GRP2, WPC)))
                        T = int(Tg2[g, p])
                        if T > 0:
                            b = int(base2[ws[0], p])
                            m_s, S_s = gather_unit(b, T, tb2p[p], OUTC,
                                                   idx2_s, s2_d, m2pool,
                                                   "msg2", "sel2")
                        for w in ws:
                            n = int(Twh2[w, p])
                            has_mm = T > 0 and n > 0
                            if not has_mm and 0 < p < NP - 1:
                                continue  # running partial unchanged
                            ps = pl2.tile([128, OUTC], f32, tag="pl2")
                            started = False
                            if has_mm:
                                t0 = int(base2[w, p]) - b
                                win_mms(ps, m_s, S_s, t0, n, True, False)
                                started = True
                            prev = own2_s if p == 0 else part_s
                            nc.tensor.matmul(ps[:], lhsT=ident_s[:],
                                             rhs=prev[:, w, :],
                                             start=not started, stop=True)
                            if p < NP - 1:
                                if (w // GRP2) % 2 == 0:
                                    nc.vector.tensor_copy(part_s[:, w, :], ps[:])
                                else:
                                    nc.scalar.activation(
                                        part_s[:, w, :], ps[:],
                                        mybir.ActivationFunctionType.Copy)
                            else:
                                o_s = smpool.tile([128, OUTC], f32, tag="o")
                                nc.scalar.activation(
                                    o_s[:], ps[:],
                                    mybir.ActivationFunctionType.Copy,
                                    scale=dc1_s[:, w : w + 1])
                                nc.sync.dma_start(out_d[w * 128 : (w + 1) * 128, :],
                                                  o_s[:])

    nc.compile()
    return nc


def kernel(x, edge_index, W1, b1, W2, b2):
    x = np.asarray(x, np.float32)
    W1 = np.asarray(W1, np.float32)
    W2 = np.asarray(W2, np.float32)
    assert not np.any(np.asarray(b1)) and not np.any(np.asarray(b2)), \
        "kernel assumes zero biases (as in the reference setup)"

    (idx1, slots1, Twh1, base1, TT1,
     idx2, slots2, Twh2, base2, TT2, dcol1, dcol2, dinv) = \
        _preprocess(np.asarray(edge_index))
    nc = _build(Twh1, base1, TT1, Twh2, base2, TT2)

    ident = np.eye(128, dtype=np.float32).astype(ml_dtypes.bfloat16)
    w1_in = np.ascontiguousarray(W1.reshape(2, 128, HID)).astype(ml_dtypes.bfloat16)
    w2_in = np.ascontiguousarray(W2.reshape(2, 128, OUTC)).astype(ml_dtypes.bfloat16)

    # canonical transposed scaled x: [2, 128, rank, RPAD]
    xd = (x * dinv[:, None]).astype(np.float32)
    xtc = np.zeros((256, NCORES, RPAD), np.float32)
    for rho in range(NCORES):
        xtc[:, rho, :RPC] = xd[rho * RPC : (rho + 1) * RPC].T
    xtc = xtc.reshape(2, 128, NCORES, RPAD).astype(ml_dtypes.bfloat16)

    in_maps = []
    for c in range(NCORES):
        rolled = np.concatenate([xtc[:, :, c:, :], xtc[:, :, :c, :]], axis=2)
        in_maps.append({
            "xtf": np.ascontiguousarray(rolled.reshape(2, 128, NROWS)),
            "w1": w1_in, "w2": w2_in, "ident": ident,
            "dcol1": dcol1[c], "dcol2": dcol2[c],
            "idx1": idx1[c], "idx2": idx2[c],
            "sel1": _sel_tiles(slots1[c], TT1),
            "sel2": _sel_tiles(slots2[c], TT2),
        })

    trace = bool(int(os.environ.get("GCN_KERNEL_TRACE", "0")))
    try:
        res = run_bass_kernel_spmd(nc, in_maps, core_ids=list(range(NCORES)), trace=trace)
    except Exception:
        # rare transient NRT exec failure: retry once on a fresh dispatch
        time_mod = __import__("time"); time_mod.sleep(2.0)
        res = run_bass_kernel_spmd(nc, in_maps, core_ids=list(range(NCORES)), trace=False)
    kernel.last_results = res
    if trace:
        print(f"HW exec time: {res.exec_time_ns} ns")
        kernel.last_exec_time_ns = res.exec_time_ns

    out = np.concatenate([res.results[c]["out"][:RPC] for c in range(NCORES)], axis=0)
    return out.astype(np.float32)


# revision 30
# speedup vs baseline: 1.2818x; 1.0895x over previous
"""GCN encoder (2-layer GCNConv, PyG-style) on 8 Trainium2 NeuronCores.

Sharding: nodes row-sharded 6250/core; edges partitioned by destination-node
owner; per-core segment-sum over 128-dst-slot windows via selection-matrix
matmuls.

Halo exchange:
  layer 1: the x @ W1 feature transform is cheap, so every core computes the
    FULL 50k-row message table itself (replicated GEMM) - no collective, and
    gathers can start as soon as the first sub-table is built.
  layer 2: the table depends on layer-1 aggregation (sharded), so it is
    all-gathered - split into two collectives (sub-tables A/B) that overlap
    with remaining gather work.

norm = dinv[src]*dinv[dst] is folded into table scaling:
  table1 = dinv .* (x @ W1)
  g~     = dinv^2 .* relu(segsum1)
  table2 = g~ @ W2
  out    = dinv .* segsum2
which is exact for b1 == 0 (the reference uses zero biases).

Self-loop messages never go through the gather path: their contribution to a
window's segment-sum is the core's own table rows, added with one identity
matmul per window from an SBUF-resident copy of the table shard.

Sub-tables (for int16 gather indices and collective splitting): local row
l < 3200 (windows 0-24) -> sub A (8*3200 = 25600 rows); l >= 3200
(windows 25-48) -> sub B (8*3072 = 24576 rows). Both < 2**15.
"""

import os
import numpy as np
import ml_dtypes

import concourse.bacc as bacc
import concourse.tile as tile
from concourse import bass, mybir
from concourse.bass_utils import run_bass_kernel_spmd
from concourse.library_config import mlp

N = 50000
INC, HID, OUTC = 256, 256, 128
NCORES = 8
RPC = N // NCORES            # 6250 rows per core
WPC = (RPC + 127) // 128     # 49 windows per core
RPAD = WPC * 128             # 6272
LSPL = 3200                  # sub-table split on local row (windows 0..24 | 25..48)
NA = NCORES * LSPL           # 25600 rows in sub-table A
NB = NCORES * (RPAD - LSPL)  # 24576 rows in sub-table B
WA = LSPL // 128             # 25 windows in A
GRP = 2                      # windows per supergather group
NGRP = (WPC + GRP - 1) // GRP
# L1 processes B-side groups first so the first AG pieces can launch early.
# group WA//GRP straddles the A/B boundary (w24,25) and is processed in the
# B phase, so windows complete in the order 24..48 then 0..23.
NGA = WA // GRP
GORDER = list(range(NGA, NGRP)) + list(range(0, NGA))

# table2 is all-gathered in four window-range pieces, listed in the order
# L1 completes them under GORDER; each fires as soon as its windows are done.
PIECES = [(24, 12), (36, 13), (0, 12), (12, 12)]
NP = len(PIECES)
GRP2 = 4                     # windows per L2 gather group
NG2 = (WPC + GRP2 - 1) // GRP2   # 13


def _win_piece(w):
    for p, (w0, wn) in enumerate(PIECES):
        if w0 <= w < w0 + wn:
            return p
    raise AssertionError(w)


def _preprocess(edge_index):
    """Edge partitioning / ordering and normalization constants (host, index-only)."""
    src = np.asarray(edge_index[0], np.int64)
    dst = np.asarray(edge_index[1], np.int64)

    # degrees include the self-loops the reference adds
    deg = (np.bincount(dst, minlength=N) + 1).astype(np.float64)
    dinv = (1.0 / np.sqrt(deg)).astype(np.float32)

    owner = dst // RPC
    dstl = dst - owner * RPC
    win = dstl >> 7
    slot = dstl & 127
    srho = src // RPC
    srl = src - srho * RPC
    sub = (srl >= LSPL).astype(np.int64)
    gl = np.where(sub == 0, srho * LSPL + srl,
                  srho * (RPAD - LSPL) + (srl - LSPL)).astype(np.int32)

    key = (owner * WPC + win) * 2 + sub
    order = np.argsort(key, kind="stable")
    key_s = key[order]
    gl_s = gl[order]
    slot_s = slot[order].astype(np.int32)

    nbuck = NCORES * WPC * 2
    counts = np.bincount(key_s, minlength=nbuck).reshape(NCORES, WPC, 2)
    starts_flat = np.concatenate([[0], np.cumsum(counts.reshape(-1))])

    # tiles per (window, sub): max over cores so one SPMD program fits all
    Twh = (counts.max(axis=0) + 127) // 128     # [WPC, 2]
    TT = int(Twh.sum())
    # stream order: group -> sub -> window in group -> tiles
    base = np.zeros((WPC, 2), np.int64)
    pos = 0
    for gi in range(NGRP):
        ws = range(gi * GRP, min((gi + 1) * GRP, WPC))
        for h in range(2):
            for w in ws:
                base[w, h] = pos
                pos += Twh[w, h]
    assert pos == TT

    idx_seq = np.zeros((NCORES, TT * 128), np.int32)
    slot_seq = np.full((NCORES, TT * 128), 128, np.int32)  # 128 = dropped sentinel
    for c in range(NCORES):
        for w in range(WPC):
            for h in range(2):
                n = counts[c, w, h]
                if n == 0:
                    continue
                s0 = starts_flat[(c * WPC + w) * 2 + h]
                p0 = base[w, h] * 128
                idx_seq[c, p0 : p0 + n] = gl_s[s0 : s0 + n]
                slot_seq[c, p0 : p0 + n] = slot_s[s0 : s0 + n]

    # wrapped int16 gather-index layout: element j at [j%16, j//16], replicated x8
    idx16 = np.empty((NCORES, 128, TT * 8), np.int16)
    slots = np.empty((NCORES, 128, TT), np.float32)
    for c in range(NCORES):
        a = idx_seq[c].astype(np.int16).reshape(-1, 16).T
        idx16[c] = np.tile(a, (8, 1))
        slots[c] = slot_seq[c].astype(np.float32).reshape(TT, 128).T

    # ---- L2 streams keyed by (dst window, src PIECE); piece-major layout.
    src_win = srl >> 7
    p2 = np.empty(len(src), np.int64)
    p2_w0 = np.empty(len(src), np.int64)
    p2_wn = np.empty(len(src), np.int64)
    for p, (w0, wn) in enumerate(PIECES):
        m = (src_win >= w0) & (src_win < w0 + wn)
        p2[m] = p
        p2_w0[m] = w0
        p2_wn[m] = wn
    row2 = (srho * (p2_wn * 128) + (srl - p2_w0 * 128)).astype(np.int32)

    key2 = (owner * WPC + win) * NP + p2
    order2 = np.argsort(key2, kind="stable")
    key2_s = key2[order2]
    row2_s = row2[order2]
    slot2_s = slot[order2].astype(np.int32)

    counts2 = np.bincount(key2_s, minlength=NCORES * WPC * NP).reshape(
        NCORES, WPC, NP)
    starts2 = np.concatenate([[0], np.cumsum(counts2.reshape(-1))])
    Twh2 = (counts2.max(axis=0) + 127) // 128   # [WPC, NP]
    base2 = np.zeros((WPC, NP), np.int64)
    pos = 0
    for p in range(NP):
        for g2 in range(NG2):
            for w in range(g2 * GRP2, min((g2 + 1) * GRP2, WPC)):
                base2[w, p] = pos
                pos += Twh2[w, p]
    TT2 = pos

    idx2_seq = np.zeros((NCORES, TT2 * 128), np.int32)
    slot2_seq = np.full((NCORES, TT2 * 128), 128, np.int32)
    for c in range(NCORES):
        for w in range(WPC):
            for p in range(NP):
                n = counts2[c, w, p]
                if n == 0:
                    continue
                s0 = starts2[(c * WPC + w) * NP + p]
                p0 = base2[w, p] * 128
                idx2_seq[c, p0 : p0 + n] = row2_s[s0 : s0 + n]
                slot2_seq[c, p0 : p0 + n] = slot2_s[s0 : s0 + n]

    idx16_2 = np.empty((NCORES, 128, TT2 * 8), np.int16)
    slots2 = np.empty((NCORES, 128, TT2), np.float32)
    for c in range(NCORES):
        a = idx2_seq[c].astype(np.int16).reshape(-1, 16).T
        idx16_2[c] = np.tile(a, (8, 1))
        slots2[c] = slot2_seq[c].astype(np.float32).reshape(TT2, 128).T

    # per-core per-window dinv columns for own rows
    dcol1 = np.zeros((NCORES, 128, WPC), np.float32)
    for c in range(NCORES):
        d = np.zeros(RPAD, np.float32)
        d[:RPC] = dinv[c * RPC : (c + 1) * RPC]
        dcol1[c] = d.reshape(WPC, 128).T
    dcol2 = dcol1 * dcol1

    return (idx16, slots, Twh, base, TT,
            idx16_2, slots2, Twh2, base2, TT2, dcol1, dcol2, dinv)


def _xt_full(x, dinv):
    """(dinv .* x)^T columns in [A | B] rank-major padded order, bf16."""
    xd = (x * dinv[:, None]).astype(np.float32)
    xt = np.zeros((256, NA + NB), np.float32)
    for rho in range(NCORES):
        xs = xd[rho * RPC : (rho + 1) * RPC].T       # [256, 6250]
        xt[:, rho * LSPL : (rho + 1) * LSPL] = xs[:, :LSPL]
        nb = RPAD - LSPL
        xt[:, NA + rho * nb : NA + rho * nb + (RPC - LSPL)] = xs[:, LSPL:]
    return np.ascontiguousarray(xt.reshape(2, 128, NA + NB)).astype(ml_dtypes.bfloat16)


def _build(TT, Twh, base, TT2, Twh2, base2):
    nc = bacc.Bacc("TRN2", num_devices=NCORES, num_swdge_queues=4)
    f32 = mybir.dt.float32
    bf = mybir.dt.bfloat16

    TBLT = (NA + NB) // 128   # 392 full-table row tiles

    xt_d = nc.dram_tensor("xtf", [2, 128, NA + NB], bf, kind="ExternalInput")
    xto_d = nc.dram_tensor("xto", [2, 128, RPAD], bf, kind="ExternalInput")
    w1_d = nc.dram_tensor("w1", [2, 128, HID], bf, kind="ExternalInput")
    w2_d = nc.dram_tensor("w2", [2, 128, OUTC], bf, kind="ExternalInput")
    iota_d = nc.dram_tensor("iota", [128, 128], bf, kind="ExternalInput")
    ident_d = nc.dram_tensor("ident", [128, 128], bf, kind="ExternalInput")
    dc1_d = nc.dram_tensor("dcol1", [128, WPC], f32, kind="ExternalInput")
    dc2_d = nc.dram_tensor("dcol2", [128, WPC], f32, kind="ExternalInput")
    idx_d = nc.dram_tensor("idx", [128, TT * 8], mybir.dt.int16, kind="ExternalInput")
    slots_d = nc.dram_tensor("slots", [128, TT], f32, kind="ExternalInput")
    idx2_d = nc.dram_tensor("idx2", [128, TT2 * 8], mybir.dt.int16, kind="ExternalInput")
    slots2_d = nc.dram_tensor("slots2", [128, TT2], f32, kind="ExternalInput")
    out_d = nc.dram_tensor("out", [RPAD, OUTC], f32, kind="ExternalOutput")

    # tiles per supergather (group, sub)
    Tg = np.zeros((NGRP, 2), np.int64)
    for gi in range(NGRP):
        ws = range(gi * GRP, min((gi + 1) * GRP, WPC))
        for h in range(2):
            Tg[gi, h] = sum(int(Twh[w, h]) for w in ws)
    Tg2 = np.zeros((NG2, NP), np.int64)
    for g2 in range(NG2):
        ws = range(g2 * GRP2, min((g2 + 1) * GRP2, WPC))
        for p in range(NP):
            Tg2[g2, p] = sum(int(Twh2[w, p]) for w in ws)

    with tile.TileContext(nc) as tc:
        nc.gpsimd.load_library(mlp)
        with (
            tc.tile_pool(name="const", bufs=1) as cpool,
            tc.tile_pool(name="gt", bufs=1) as gtpool,
            tc.tile_pool(name="xts", bufs=3) as xtpool,
            tc.tile_pool(name="evac", bufs=3) as epool,
            tc.tile_pool(name="msg", bufs=5) as mpool,
            tc.tile_pool(name="sel", bufs=3) as spool,
            tc.tile_pool(name="part", bufs=WPC + 8) as ppool,
            tc.tile_pool(name="p256", bufs=4, space="PSUM") as p256,
            tc.tile_pool(name="p128", bufs=2, space="PSUM") as p128,
            tc.tile_pool(name="ptr", bufs=2, space="PSUM") as ptr,
            tc.tile_pool(name="dram", bufs=1, space="DRAM") as dram,
        ):
            # ---- constants to SBUF
            w1_s = cpool.tile([128, 2, HID], bf)
            w2_s = cpool.tile([128, 2, OUTC], bf)
            iota_s = cpool.tile([128, 128], bf)
            ident_s = cpool.tile([128, 128], bf)
            dc1_s = cpool.tile([128, WPC], f32)
            dc2_s = cpool.tile([128, WPC], f32)
            idx_s = cpool.tile([128, TT * 8], mybir.dt.int16)
            slots_s = cpool.tile([128, TT], f32)
            idx2_s = cpool.tile([128, TT2 * 8], mybir.dt.int16)
            slots2_s = cpool.tile([128, TT2], f32)
            gt_s = gtpool.tile([128, 2, RPAD], bf)      # g~^T  [ch%128, ch//128, row]
            own1_s = gtpool.tile([128, WPC, HID], bf)   # own table1 rows
            own2_s = gtpool.tile([128, WPC, OUTC], bf)  # own table2 rows
            for k in range(2):
                nc.sync.dma_start(w1_s[:, k, :], w1_d[k])
                nc.sync.dma_start(w2_s[:, k, :], w2_d[k])
            nc.sync.dma_start(iota_s[:], iota_d[:])
            nc.sync.dma_start(ident_s[:], ident_d[:])
            nc.sync.dma_start(dc1_s[:], dc1_d[:])
            nc.sync.dma_start(dc2_s[:], dc2_d[:])
            nc.sync.dma_start(idx_s[:], idx_d[:])
            nc.sync.dma_start(slots_s[:], slots_d[:])
            nc.sync.dma_start(idx2_s[:], idx2_d[:])
            nc.sync.dma_start(slots2_s[:], slots2_d[:])

            tb1a = dram.tile([NA, HID], bf)
            tb1b = dram.tile([NB, HID], bf)
            ag_in = [dram.tile([wn * 128, OUTC], bf, name=f"ag_in{p}")
                     for p, (_, wn) in enumerate(PIECES)]
            tb2p = [dram.tile([NCORES * wn * 128, OUTC], bf, name=f"tb2p{p}")
                    for p, (_, wn) in enumerate(PIECES)]

            CH = 8  # row-tiles per xt chunk
            # ---- own rows pass (feeds the per-window self-loop matmul)
            with nc.named_scope("p1own"):
                for c0 in range(0, WPC, CH):
                    cn = min(CH, WPC - c0)
                    xt_t = xtpool.tile([128, 2, CH * 128], bf, tag="xt")
                    nc.scalar.dma_start(
                        xt_t[:, :, : cn * 128],
                        xto_d[:, :, c0 * 128 : (c0 + cn) * 128].rearrange("k p n -> p k n"))
                    for j in range(cn):
                        rt = c0 + j
                        ps = p256.tile([128, HID], f32, tag="p256")
                        for k in range(2):
                            nc.tensor.matmul(ps[:], lhsT=xt_t[:, k, j * 128 : (j + 1) * 128],
                                             rhs=w1_s[:, k, :], start=(k == 0), stop=(k == 1))
                        nc.vector.tensor_copy(own1_s[:, rt, :], ps[:])

            # ---- phase 1: replicated table1 = (dinv .* x) @ W1 (dinv folded on host)
            def gemm_tiles(t0, t1, tb, roff):
                for c0 in range(t0, t1, CH):
                    cn = min(CH, t1 - c0)
                    xt_t = xtpool.tile([128, 2, CH * 128], bf, tag="xt")
                    nc.scalar.dma_start(
                        xt_t[:, :, : cn * 128],
                        xt_d[:, :, c0 * 128 : (c0 + cn) * 128].rearrange("k p n -> p k n"),
                    )
                    ev = epool.tile([128, CH, HID], bf, tag="xw")
                    for j in range(cn):
                        pool_j = p256 if j % 2 == 0 else p128
                        ps = pool_j.tile([128, HID], f32, tag="p256" if j % 2 == 0 else "p128")
                        for k in range(2):
                            nc.tensor.matmul(
                                ps[:], lhsT=xt_t[:, k, j * 128 : (j + 1) * 128],
                                rhs=w1_s[:, k, :], start=(k == 0), stop=(k == 1))
                        if j % 2 == 0:
                            nc.vector.tensor_copy(ev[:, j, :], ps[:])
                        else:
                            nc.scalar.activation(ev[:, j, :], ps[:],
                                                 mybir.ActivationFunctionType.Copy)
                    r0 = (c0 - t0) * 128 + roff
                    nc.sync.dma_start(
                        tb[r0 : r0 + cn * 128, :].rearrange("(j p) c -> p j c", p=128),
                        ev[:, :cn, :])

            with nc.named_scope("p1a"):
                gemm_tiles(0, NA // 128, tb1a, 0)
            with nc.named_scope("p1b"):
                gemm_tiles(NA // 128, TBLT, tb1b, 0)

            # ---- edge aggregation unit: gathers + S build + matmuls for one
            #      (group, sub); psum handling differs per layer
            def gather_unit(gi, h, tbl, width, qctr):
                T = int(Tg[gi, h])
                if T == 0:
                    return None, None
                ws = list(range(gi * GRP, min((gi + 1) * GRP, WPC)))
                b = int(base[ws[0], h])
                m_s = mpool.tile([128, T, width], bf, tag="msg")
                nc.gpsimd.dma_gather(
                    m_s[:], tbl[:, :], idx_s[:, b * 8 : (b + T) * 8],
                    T * 128, T * 128, width,
                    single_packet=False, queue_num=qctr[0] % 4)
                qctr[0] += 1
                S_s = spool.tile([128, T, 128], bf, tag="sel")
                nc.vector.tensor_tensor(
                    out=S_s[:],
                    in0=slots_s[:, b : b + T, None].to_broadcast([128, T, 128]),
                    in1=iota_s[:, None, :].to_broadcast([128, T, 128]),
                    op=mybir.AluOpType.is_equal)
                return m_s, S_s

            def win_mms(w, h, ps, m_s, S_s, first, last):
                # base of this unit's stream is base[first window of group, h]
                gw0 = (w // GRP) * GRP
                b = int(base[gw0, h])
                n = int(Twh[w, h])
                for t in range(n):
                    tt = int(base[w, h]) - b + t
                    nc.tensor.matmul(ps[:], lhsT=S_s[:, tt, :], rhs=m_s[:, tt, :],
                                     start=(first and t == 0),
                                     stop=(last and t == n - 1))

            qctr = [0]

            # ---- phase 3: layer-1 aggregation (B-side groups first)
            def l1_group(gi):
                ws = list(range(gi * GRP, min((gi + 1) * GRP, WPC)))
                units = {}
                for h in range(2):
                    units[h] = gather_unit(gi, h, tb1a if h == 0 else tb1b, HID, qctr)
                pss = {}
                for w in ws:
                    ps = p256.tile([128, HID], f32, tag="p256")
                    pss[w] = ps
                    started = False
                    for h in range(2):
                        m_s, S_s = units[h]
                        if m_s is None or Twh[w, h] == 0:
                            continue
                        win_mms(w, h, ps, m_s, S_s, not started, False)
                        started = True
                    # self-loop contribution last (own1_s ready late is fine)
                    nc.tensor.matmul(ps[:], lhsT=ident_s[:], rhs=own1_s[:, w, :],
                                     start=not started, stop=True)
                for w in ws:
                    ps = pss[w]
                    g_s = epool.tile([128, HID], bf, tag="g")
                    nc.scalar.activation(g_s[:], ps[:], mybir.ActivationFunctionType.Relu,
                                         scale=dc2_s[:, w : w + 1])
                    for k in range(2):
                        pt = ptr.tile([128, 128], bf, tag="pt")
                        nc.tensor.transpose(pt[:], g_s[:, k * 128 : (k + 1) * 128],
                                            ident_s[:])
                        nc.vector.tensor_copy(gt_s[:, k, w * 128 : (w + 1) * 128], pt[:])
                    ps2 = p128.tile([128, OUTC], f32, tag="p128")
                    for k in range(2):
                        nc.tensor.matmul(ps2[:], lhsT=gt_s[:, k, w * 128 : (w + 1) * 128],
                                         rhs=w2_s[:, k, :], start=(k == 0), stop=(k == 1))
                    nc.vector.tensor_copy(own2_s[:, w, :], ps2[:])
                    p = _win_piece(w)
                    w0 = PIECES[p][0]
                    nc.sync.dma_start(
                        ag_in[p][(w - w0) * 128 : (w - w0 + 1) * 128, :],
                        own2_s[:, w, :])

            # ---- phase 3: layer-1 aggregation; AG piece p fires as soon as
            # all its windows have been evacuated (GORDER completes pieces in
            # PIECES order).
            done_w = set()
            fired = set()
            with nc.named_scope("p3"):
                for gi in GORDER:
                    l1_group(gi)
                    done_w.update(range(gi * GRP, min((gi + 1) * GRP, WPC)))
                    for p, (w0, wn) in enumerate(PIECES):
                        if p not in fired and all(
                                w in done_w for w in range(w0, w0 + wn)):
                            fired.add(p)
                            with nc.named_scope(f"ag{p}"):
                                nc.gpsimd.collective_compute(
                                    "AllGather", mybir.AluOpType.bypass,
                                    replica_groups=[list(range(NCORES))],
                                    ins=[ag_in[p].opt()], outs=[tb2p[p].opt()])
            assert len(fired) == NP

            # ---- L2 gather unit over the (group, piece) streams
            def gather_unit2(g2, p, width):
                T = int(Tg2[g2, p])
                if T == 0:
                    return None, None
                ws = list(range(g2 * GRP2, min((g2 + 1) * GRP2, WPC)))
                b = int(base2[ws[0], p])
                m_s = mpool.tile([128, T, width], bf, tag="msg")
                nc.gpsimd.dma_gather(
                    m_s[:], tb2p[p][:, :], idx2_s[:, b * 8 : (b + T) * 8],
                    T * 128, T * 128, width,
                    single_packet=False, queue_num=qctr[0] % 4)
                qctr[0] += 1
                S_s = spool.tile([128, T, 128], bf, tag="sel")
                nc.vector.tensor_tensor(
                    out=S_s[:],
                    in0=slots2_s[:, b : b + T, None].to_broadcast([128, T, 128]),
                    in1=iota_s[:, None, :].to_broadcast([128, T, 128]),
                    op=mybir.AluOpType.is_equal)
                return m_s, S_s

            def win_mms2(w, p, ps, m_s, S_s, first, last):
                gw0 = (w // GRP2) * GRP2
                b = int(base2[gw0, p])
                n = int(Twh2[w, p])
                for t in range(n):
                    tt = int(base2[w, p]) - b + t
                    nc.tensor.matmul(ps[:], lhsT=S_s[:, tt, :], rhs=m_s[:, tt, :],
                                     start=(first and t == 0),
                                     stop=(last and t == n - 1))

            # ---- phase 6: layer-2 aggregation, piece-major passes; bf16
            # running partials; last piece scales and writes output.
            partials = {}
            for p in range(NP):
                with nc.named_scope(f"p6_{p}"):
                    for g2 in range(NG2):
                        ws = list(range(g2 * GRP2, min((g2 + 1) * GRP2, WPC)))
                        m_s, S_s = gather_unit2(g2, p, OUTC)
                        for w in ws:
                            has = m_s is not None and Twh2[w, p] > 0
                            if p == 0:
                                ps = p256.tile([128, OUTC], f32, tag="p256")
                                started = False
                                if has:
                                    win_mms2(w, p, ps, m_s, S_s, True, False)
                                    started = True
                                nc.tensor.matmul(ps[:], lhsT=ident_s[:],
                                                 rhs=own2_s[:, w, :],
                                                 start=not started, stop=True)
                                pp = ppool.tile([128, OUTC], bf, tag="partial")
                                nc.vector.tensor_copy(pp[:], ps[:])
                                partials[w] = pp
                            elif p < NP - 1:
                                if not has:
                                    continue  # partial unchanged
                                ps = p256.tile([128, OUTC], f32, tag="p256")
                                win_mms2(w, p, ps, m_s, S_s, True, True)
                                ppn = ppool.tile([128, OUTC], bf, tag="partial")
                                nc.vector.tensor_add(ppn[:], ps[:], partials[w][:])
                                partials[w] = ppn
                            else:
                                o_s = epool.tile([128, OUTC], f32, tag="o")
                                if has:
                                    ps = p256.tile([128, OUTC], f32, tag="p256")
                                    win_mms2(w, p, ps, m_s, S_s, True, True)
                                    acc = epool.tile([128, OUTC], f32, tag="acc")
                                    nc.vector.tensor_add(acc[:], ps[:],
                                                         partials[w][:])
                                else:
                                    acc = partials[w]
                                nc.scalar.activation(o_s[:], acc[:],
                                                     mybir.ActivationFunctionType.Copy,
                                                     scale=dc1_s[:, w : w + 1])
                                nc.sync.dma_start(out_d[w * 128 : (w + 1) * 128, :],
                                                  o_s[:])

    nc.compile()
    return nc


def kernel(x, edge_index, W1, b1, W2, b2):
    x = np.asarray(x, np.float32)
    W1 = np.asarray(W1, np.float32)
    W2 = np.asarray(W2, np.float32)
    assert not np.any(np.asarray(b1)) and not np.any(np.asarray(b2)), \
        "kernel assumes zero biases (as in the reference setup)"

    (idx16, slots, Twh, base, TT,
     idx16_2, slots2, Twh2, base2, TT2, dcol1, dcol2, dinv) = \
        _preprocess(np.asarray(edge_index))
    nc = _build(TT, Twh, base, TT2, Twh2, base2)

    iota = np.broadcast_to(np.arange(128, dtype=np.float32), (128, 128)).astype(ml_dtypes.bfloat16)
    ident = np.eye(128, dtype=np.float32).astype(ml_dtypes.bfloat16)
    w1_in = np.ascontiguousarray(W1.reshape(2, 128, HID)).astype(ml_dtypes.bfloat16)
    w2_in = np.ascontiguousarray(W2.reshape(2, 128, OUTC)).astype(ml_dtypes.bfloat16)
    xtf = _xt_full(x, dinv)

    xd = (x * dinv[:, None]).astype(np.float32)
    in_maps = []
    for c in range(NCORES):
        xto = np.zeros((256, RPAD), np.float32)
        xto[:, :RPC] = xd[c * RPC : (c + 1) * RPC].T
        in_maps.append({
            "xtf": xtf,
            "xto": np.ascontiguousarray(xto.reshape(2, 128, RPAD)).astype(ml_dtypes.bfloat16),
            "w1": w1_in, "w2": w2_in, "iota": iota, "ident": ident,
            "dcol1": dcol1[c], "dcol2": dcol2[c],
            "idx": idx16[c], "slots": slots[c],
            "idx2": idx16_2[c], "slots2": slots2[c],
        })

    trace = bool(int(os.environ.get("GCN_KERNEL_TRACE", "0")))
    try:
        res = run_bass_kernel_spmd(nc, in_maps, core_ids=list(range(NCORES)), trace=trace)
    except Exception:
        # rare transient NRT exec failure: retry once on a fresh dispatch
        time_mod = __import__("time"); time_mod.sleep(2.0)
        res = run_bass_kernel_spmd(nc, in_maps, core_ids=list(range(NCORES)), trace=False)
    kernel.last_results = res
    if trace:
        print(f"HW exec time: {res.exec_time_ns} ns")
        kernel.last_exec_time_ns = res.exec_time_ns

    out = np.concatenate([res.results[c]["out"][:RPC] for c in range(NCORES)], axis=0)
    return out.astype(np.float32)



# revision 33
# speedup vs baseline: 1.8178x; 1.4181x over previous
"""GCN encoder (2-layer GCNConv, PyG-style) on 8 Trainium2 NeuronCores.

Sharding: nodes row-sharded 6250/core; edges partitioned by destination-node
owner; per-core segment-sum over 128-dst-slot windows via selection-matrix
matmuls.

Halo exchange:
  layer 1: the x @ W1 feature transform is cheap, so every core computes the
    FULL 50k-row message table itself (replicated GEMM) - no collective, and
    gathers can start as soon as the first sub-table is built.
  layer 2: the table depends on layer-1 aggregation (sharded), so it is
    all-gathered - split into two collectives (sub-tables A/B) that overlap
    with remaining gather work.

norm = dinv[src]*dinv[dst] is folded into table scaling:
  table1 = dinv .* (x @ W1)
  g~     = dinv^2 .* relu(segsum1)
  table2 = g~ @ W2
  out    = dinv .* segsum2
which is exact for b1 == 0 (the reference uses zero biases).

Self-loop messages never go through the gather path: their contribution to a
window's segment-sum is the core's own table rows, added with one identity
matmul per window from an SBUF-resident copy of the table shard.

Sub-tables (for int16 gather indices and collective splitting): local row
l < 3200 (windows 0-24) -> sub A (8*3200 = 25600 rows); l >= 3200
(windows 25-48) -> sub B (8*3072 = 24576 rows). Both < 2**15.
"""

import os
import numpy as np
import ml_dtypes

import concourse.bacc as bacc
import concourse.tile as tile
from concourse import bass, mybir
from concourse.bass_utils import run_bass_kernel_spmd
from concourse.library_config import mlp

N = 50000
INC, HID, OUTC = 256, 256, 128
NCORES = 8
RPC = N // NCORES            # 6250 rows per core
WPC = (RPC + 127) // 128     # 49 windows per core
RPAD = WPC * 128             # 6272
LSPL = 2176                  # sub-table split on local row (windows 0..16 | 17..48)
NA = NCORES * LSPL           # 17408 rows in sub-table A
NB = NCORES * (RPAD - LSPL)  # 32768 rows in sub-table B (max idx 32767: int16 limit)
WA = LSPL // 128             # 17 windows in A
# Asymmetric on purpose: sub-A is the LAST all-gather to fire (its windows
# finish last under GORDER), so making it small shrinks the serial tail
# (smaller final collective + less layer-2 work gated on it).
GRP = 2                      # windows per supergather group
NGRP = (WPC + GRP - 1) // GRP
# L1 processes B-side groups first so AG2(B) can launch early.
# window 24 (the last A-side window) sits in group 8 = ceil(25/3)-1.
# first group that contains any B-side window (w>=25); group WA//GRP
# straddles the boundary (w24,25,26) and is processed in the B phase,
# so after the B phase windows 24..48 are all evacuated.
NGA = WA // GRP
GORDER = list(range(NGA, NGRP)) + list(range(0, NGA))


def _preprocess(edge_index):
    """Edge partitioning / ordering and normalization constants (host, index-only)."""
    src = np.asarray(edge_index[0], np.int64)
    dst = np.asarray(edge_index[1], np.int64)

    # degrees include the self-loops the reference adds
    deg = (np.bincount(dst, minlength=N) + 1).astype(np.float64)
    dinv = (1.0 / np.sqrt(deg)).astype(np.float32)

    owner = dst // RPC
    dstl = dst - owner * RPC
    win = dstl >> 7
    slot = dstl & 127
    srho = src // RPC
    srl = src - srho * RPC
    sub = (srl >= LSPL).astype(np.int64)
    gl = np.where(sub == 0, srho * LSPL + srl,
                  srho * (RPAD - LSPL) + (srl - LSPL)).astype(np.int32)

    key = (owner * WPC + win) * 2 + sub
    order = np.argsort(key, kind="stable")
    key_s = key[order]
    gl_s = gl[order]
    slot_s = slot[order].astype(np.int32)

    nbuck = NCORES * WPC * 2
    counts = np.bincount(key_s, minlength=nbuck).reshape(NCORES, WPC, 2)
    starts_flat = np.concatenate([[0], np.cumsum(counts.reshape(-1))])

    # tiles per (window, sub): max over cores so one SPMD program fits all
    Twh = (counts.max(axis=0) + 127) // 128     # [WPC, 2]
    TT = int(Twh.sum())
    # stream order: group -> sub -> window in group -> tiles
    base = np.zeros((WPC, 2), np.int64)
    pos = 0
    for gi in range(NGRP):
        ws = range(gi * GRP, min((gi + 1) * GRP, WPC))
        for h in range(2):
            for w in ws:
                base[w, h] = pos
                pos += Twh[w, h]
    assert pos == TT

    idx_seq = np.zeros((NCORES, TT * 128), np.int32)
    slot_seq = np.full((NCORES, TT * 128), 128, np.int32)  # 128 = dropped sentinel
    for c in range(NCORES):
        for w in range(WPC):
            for h in range(2):
                n = counts[c, w, h]
                if n == 0:
                    continue
                s0 = starts_flat[(c * WPC + w) * 2 + h]
                p0 = base[w, h] * 128
                idx_seq[c, p0 : p0 + n] = gl_s[s0 : s0 + n]
                slot_seq[c, p0 : p0 + n] = slot_s[s0 : s0 + n]

    # wrapped int16 gather-index layout: element j at [j%16, j//16], replicated x8
    idx16 = np.empty((NCORES, 128, TT * 8), np.int16)
    slots = np.empty((NCORES, 128, TT), np.float32)
    for c in range(NCORES):
        a = idx_seq[c].astype(np.int16).reshape(-1, 16).T
        idx16[c] = np.tile(a, (8, 1))
        slots[c] = slot_seq[c].astype(np.float32).reshape(TT, 128).T

    # per-core per-window dinv columns for own rows
    dcol1 = np.zeros((NCORES, 128, WPC), np.float32)
    for c in range(NCORES):
        d = np.zeros(RPAD, np.float32)
        d[:RPC] = dinv[c * RPC : (c + 1) * RPC]
        dcol1[c] = d.reshape(WPC, 128).T
    dcol2 = dcol1 * dcol1

    return idx16, slots, Twh, base, TT, dcol1, dcol2, dinv


def _xt_full(x, dinv):
    """(dinv .* x)^T columns in [A | B] rank-major padded order, bf16."""
    xd = (x * dinv[:, None]).astype(np.float32)
    xt = np.zeros((256, NA + NB), np.float32)
    for rho in range(NCORES):
        xs = xd[rho * RPC : (rho + 1) * RPC].T       # [256, 6250]
        xt[:, rho * LSPL : (rho + 1) * LSPL] = xs[:, :LSPL]
        nb = RPAD - LSPL
        xt[:, NA + rho * nb : NA + rho * nb + (RPC - LSPL)] = xs[:, LSPL:]
    return np.ascontiguousarray(xt.reshape(2, 128, NA + NB)).astype(ml_dtypes.bfloat16)


def _build(TT, Twh, base):
    nc = bacc.Bacc("TRN2", num_devices=NCORES, num_swdge_queues=4)
    f32 = mybir.dt.float32
    bf = mybir.dt.bfloat16

    TBLT = (NA + NB) // 128   # 392 full-table row tiles

    xt_d = nc.dram_tensor("xtf", [2, 128, NA + NB], bf, kind="ExternalInput")
    xto_d = nc.dram_tensor("xto", [2, 128, RPAD], bf, kind="ExternalInput")
    w1_d = nc.dram_tensor("w1", [2, 128, HID], bf, kind="ExternalInput")
    w2_d = nc.dram_tensor("w2", [2, 128, OUTC], bf, kind="ExternalInput")
    iota_d = nc.dram_tensor("iota", [128, 128], bf, kind="ExternalInput")
    ident_d = nc.dram_tensor("ident", [128, 128], bf, kind="ExternalInput")
    dc1_d = nc.dram_tensor("dcol1", [128, WPC], f32, kind="ExternalInput")
    dc2_d = nc.dram_tensor("dcol2", [128, WPC], f32, kind="ExternalInput")
    idx_d = nc.dram_tensor("idx", [128, TT * 8], mybir.dt.int16, kind="ExternalInput")
    slots_d = nc.dram_tensor("slots", [128, TT], f32, kind="ExternalInput")
    out_d = nc.dram_tensor("out", [RPAD, OUTC], f32, kind="ExternalOutput")

    # tiles per supergather (group, sub)
    Tg = np.zeros((NGRP, 2), np.int64)
    for gi in range(NGRP):
        ws = range(gi * GRP, min((gi + 1) * GRP, WPC))
        for h in range(2):
            Tg[gi, h] = sum(int(Twh[w, h]) for w in ws)

    with tile.TileContext(nc) as tc:
        nc.gpsimd.load_library(mlp)
        with (
            tc.tile_pool(name="const", bufs=1) as cpool,
            tc.tile_pool(name="gt", bufs=1) as gtpool,
            tc.tile_pool(name="xts", bufs=3) as xtpool,
            tc.tile_pool(name="evac", bufs=3) as epool,
            tc.tile_pool(name="msg", bufs=5) as mpool,
            tc.tile_pool(name="sel", bufs=4) as spool,
            tc.tile_pool(name="part", bufs=WPC) as ppool,
            tc.tile_pool(name="p256", bufs=4, space="PSUM") as p256,
            tc.tile_pool(name="p128", bufs=2, space="PSUM") as p128,
            tc.tile_pool(name="ptr", bufs=2, space="PSUM") as ptr,
            tc.tile_pool(name="dram", bufs=1, space="DRAM") as dram,
        ):
            # ---- constants to SBUF
            w1_s = cpool.tile([128, 2, HID], bf)
            w2_s = cpool.tile([128, 2, OUTC], bf)
            iota_s = cpool.tile([128, 128], bf)
            ident_s = cpool.tile([128, 128], bf)
            dc1_s = cpool.tile([128, WPC], f32)
            dc2_s = cpool.tile([128, WPC], f32)
            idx_s = cpool.tile([128, TT * 8], mybir.dt.int16)
            slots_s = cpool.tile([128, TT], f32)
            gt_s = gtpool.tile([128, 2, RPAD], bf)      # g~^T  [ch%128, ch//128, row]
            own1_s = gtpool.tile([128, WPC, HID], bf)   # own table1 rows
            own2_s = gtpool.tile([128, WPC, OUTC], bf)  # own table2 rows
            for k in range(2):
                nc.sync.dma_start(w1_s[:, k, :], w1_d[k])
                nc.sync.dma_start(w2_s[:, k, :], w2_d[k])
            nc.sync.dma_start(iota_s[:], iota_d[:])
            nc.sync.dma_start(ident_s[:], ident_d[:])
            nc.sync.dma_start(dc1_s[:], dc1_d[:])
            nc.sync.dma_start(dc2_s[:], dc2_d[:])
            nc.sync.dma_start(idx_s[:], idx_d[:])
            nc.sync.dma_start(slots_s[:], slots_d[:])

            tb1a = dram.tile([NA, HID], bf)
            tb1b = dram.tile([NB, HID], bf)
            ag2a_in = dram.tile([LSPL, OUTC], bf)
            ag2b_in = dram.tile([RPAD - LSPL, OUTC], bf)
            tb2a = dram.tile([NA, OUTC], bf)
            tb2b = dram.tile([NB, OUTC], bf)

            CH = 8  # row-tiles per xt chunk
            # ---- own rows pass (feeds the per-window self-loop matmul)
            with nc.named_scope("p1own"):
                for c0 in range(0, WPC, CH):
                    cn = min(CH, WPC - c0)
                    xt_t = xtpool.tile([128, 2, CH * 128], bf, tag="xt")
                    nc.scalar.dma_start(
                        xt_t[:, :, : cn * 128],
                        xto_d[:, :, c0 * 128 : (c0 + cn) * 128].rearrange("k p n -> p k n"))
                    for j in range(cn):
                        rt = c0 + j
                        ps = p256.tile([128, HID], f32, tag="p256")
                        for k in range(2):
                            nc.tensor.matmul(ps[:], lhsT=xt_t[:, k, j * 128 : (j + 1) * 128],
                                             rhs=w1_s[:, k, :], start=(k == 0), stop=(k == 1))
                        nc.vector.tensor_copy(own1_s[:, rt, :], ps[:])

            # ---- phase 1: replicated table1 = (dinv .* x) @ W1 (dinv folded on host)
            def gemm_tiles(t0, t1, tb, roff):
                for c0 in range(t0, t1, CH):
                    cn = min(CH, t1 - c0)
                    xt_t = xtpool.tile([128, 2, CH * 128], bf, tag="xt")
                    nc.scalar.dma_start(
                        xt_t[:, :, : cn * 128],
                        xt_d[:, :, c0 * 128 : (c0 + cn) * 128].rearrange("k p n -> p k n"),
                    )
                    ev = epool.tile([128, CH, HID], bf, tag="xw")
                    for j in range(cn):
                        pool_j = p256 if j % 2 == 0 else p128
                        ps = pool_j.tile([128, HID], f32, tag="p256" if j % 2 == 0 else "p128")
                        for k in range(2):
                            nc.tensor.matmul(
                                ps[:], lhsT=xt_t[:, k, j * 128 : (j + 1) * 128],
                                rhs=w1_s[:, k, :], start=(k == 0), stop=(k == 1))
                        if j % 2 == 0:
                            nc.vector.tensor_copy(ev[:, j, :], ps[:])
                        else:
                            nc.scalar.activation(ev[:, j, :], ps[:],
                                                 mybir.ActivationFunctionType.Copy)
                    r0 = (c0 - t0) * 128 + roff
                    nc.sync.dma_start(
                        tb[r0 : r0 + cn * 128, :].rearrange("(j p) c -> p j c", p=128),
                        ev[:, :cn, :])

            with nc.named_scope("p1a"):
                gemm_tiles(0, NA // 128, tb1a, 0)
            with nc.named_scope("p1b"):
                gemm_tiles(NA // 128, TBLT, tb1b, 0)

            # ---- edge aggregation unit: gathers + S build + matmuls for one
            #      (group, sub); psum handling differs per layer
            def gather_unit(gi, h, tbl, width, qctr):
                T = int(Tg[gi, h])
                if T == 0:
                    return None, None
                ws = list(range(gi * GRP, min((gi + 1) * GRP, WPC)))
                b = int(base[ws[0], h])
                m_s = mpool.tile([128, T, width], bf, tag="msg")
                nc.gpsimd.dma_gather(
                    m_s[:], tbl[:, :], idx_s[:, b * 8 : (b + T) * 8],
                    T * 128, T * 128, width,
                    single_packet=False, queue_num=qctr[0] % 4)
                qctr[0] += 1
                S_s = spool.tile([128, T, 128], bf, tag="sel")
                nc.vector.tensor_tensor(
                    out=S_s[:],
                    in0=slots_s[:, b : b + T, None].to_broadcast([128, T, 128]),
                    in1=iota_s[:, None, :].to_broadcast([128, T, 128]),
                    op=mybir.AluOpType.is_equal)
                return m_s, S_s

            def win_mms(w, h, ps, m_s, S_s, first, last):
                # base of this unit's stream is base[first window of group, h]
                gw0 = (w // GRP) * GRP
                b = int(base[gw0, h])
                n = int(Twh[w, h])
                for t in range(n):
                    tt = int(base[w, h]) - b + t
                    nc.tensor.matmul(ps[:], lhsT=S_s[:, tt, :], rhs=m_s[:, tt, :],
                                     start=(first and t == 0),
                                     stop=(last and t == n - 1))

            qctr = [0]

            # ---- phase 3: layer-1 aggregation (B-side groups first)
            def l1_group(gi):
                ws = list(range(gi * GRP, min((gi + 1) * GRP, WPC)))
                units = {}
                for h in range(2):
                    units[h] = gather_unit(gi, h, tb1a if h == 0 else tb1b, HID, qctr)
                pss = {}
                for w in ws:
                    ps = p256.tile([128, HID], f32, tag="p256")
                    pss[w] = ps
                    started = False
                    for h in range(2):
                        m_s, S_s = units[h]
                        if m_s is None or Twh[w, h] == 0:
                            continue
                        win_mms(w, h, ps, m_s, S_s, not started, False)
                        started = True
                    # self-loop contribution last (own1_s ready late is fine)
                    nc.tensor.matmul(ps[:], lhsT=ident_s[:], rhs=own1_s[:, w, :],
                                     start=not started, stop=True)
                for w in ws:
                    ps = pss[w]
                    g_s = epool.tile([128, HID], bf, tag="g")
                    nc.scalar.activation(g_s[:], ps[:], mybir.ActivationFunctionType.Relu,
                                         scale=dc2_s[:, w : w + 1])
                    for k in range(2):
                        pt = ptr.tile([128, 128], bf, tag="pt")
                        nc.tensor.transpose(pt[:], g_s[:, k * 128 : (k + 1) * 128],
                                            ident_s[:])
                        nc.vector.tensor_copy(gt_s[:, k, w * 128 : (w + 1) * 128], pt[:])
                    ps2 = p128.tile([128, OUTC], f32, tag="p128")
                    for k in range(2):
                        nc.tensor.matmul(ps2[:], lhsT=gt_s[:, k, w * 128 : (w + 1) * 128],
                                         rhs=w2_s[:, k, :], start=(k == 0), stop=(k == 1))
                    nc.vector.tensor_copy(own2_s[:, w, :], ps2[:])
                    if w < WA:
                        nc.sync.dma_start(ag2a_in[w * 128 : (w + 1) * 128, :],
                                          own2_s[:, w, :])
                    else:
                        nc.sync.dma_start(ag2b_in[(w - WA) * 128 : (w - WA + 1) * 128, :],
                                          own2_s[:, w, :])

            with nc.named_scope("p3_l1b"):
                for gi in GORDER[: NGRP - NGA]:
                    l1_group(gi)
            # AG2 for sub-table B launches while L1 still works on A-side groups
            with nc.named_scope("ag2b"):
                nc.gpsimd.collective_compute(
                    "AllGather", mybir.AluOpType.bypass,
                    replica_groups=[list(range(NCORES))],
                    ins=[ag2b_in.opt()], outs=[tb2b.opt()])
            with nc.named_scope("p3_l1a"):
                for gi in GORDER[NGRP - NGA :]:
                    l1_group(gi)
            with nc.named_scope("ag2a"):
                nc.gpsimd.collective_compute(
                    "AllGather", mybir.AluOpType.bypass,
                    replica_groups=[list(range(NCORES))],
                    ins=[ag2a_in.opt()], outs=[tb2a.opt()])

            # ---- phase 6: layer-2 aggregation, two stages so AG latency hides
            partials = {}
            with nc.named_scope("p6_b"):
                # stage 1: self + sub-B messages -> partial (frees PSUM quickly)
                for gi in range(NGRP):
                    ws = list(range(gi * GRP, min((gi + 1) * GRP, WPC)))
                    m_s, S_s = gather_unit(gi, 1, tb2b, OUTC, qctr)
                    for w in ws:
                        ps = p256.tile([128, OUTC], f32, tag="p256")
                        started = False
                        if m_s is not None and Twh[w, 1] > 0:
                            win_mms(w, 1, ps, m_s, S_s, True, False)
                            started = True
                        nc.tensor.matmul(ps[:], lhsT=ident_s[:], rhs=own2_s[:, w, :],
                                         start=not started, stop=True)
                        pp = ppool.tile([128, OUTC], bf, tag="partial")
                        nc.vector.tensor_copy(pp[:], ps[:])
                        partials[w] = pp
            with nc.named_scope("p6_a"):
                # stage 2: sub-A messages + partial -> output
                for gi in range(NGRP):
                    ws = list(range(gi * GRP, min((gi + 1) * GRP, WPC)))
                    m_s, S_s = gather_unit(gi, 0, tb2a, OUTC, qctr)
                    for w in ws:
                        o_s = epool.tile([128, OUTC], f32, tag="o")
                        if m_s is not None and Twh[w, 0] > 0:
                            ps = p256.tile([128, OUTC], f32, tag="p256")
                            win_mms(w, 0, ps, m_s, S_s, True, True)
                            acc = epool.tile([128, OUTC], f32, tag="acc")
                            nc.vector.tensor_add(acc[:], ps[:], partials[w][:])
                        else:
                            acc = partials[w]
                        nc.scalar.activation(o_s[:], acc[:],
                                             mybir.ActivationFunctionType.Copy,
                                             scale=dc1_s[:, w : w + 1])
                        nc.sync.dma_start(out_d[w * 128 : (w + 1) * 128, :], o_s[:])

    nc.compile()
    return nc


def kernel(x, edge_index, W1, b1, W2, b2):
    x = np.asarray(x, np.float32)
    W1 = np.asarray(W1, np.float32)
    W2 = np.asarray(W2, np.float32)
    assert not np.any(np.asarray(b1)) and not np.any(np.asarray(b2)), \
        "kernel assumes zero biases (as in the reference setup)"

    idx16, slots, Twh, base, TT, dcol1, dcol2, dinv = _preprocess(np.asarray(edge_index))
    nc = _build(TT, Twh, base)

    iota = np.broadcast_to(np.arange(128, dtype=np.float32), (128, 128)).astype(ml_dtypes.bfloat16)
    ident = np.eye(128, dtype=np.float32).astype(ml_dtypes.bfloat16)
    w1_in = np.ascontiguousarray(W1.reshape(2, 128, HID)).astype(ml_dtypes.bfloat16)
    w2_in = np.ascontiguousarray(W2.reshape(2, 128, OUTC)).astype(ml_dtypes.bfloat16)
    xtf = _xt_full(x, dinv)

    xd = (x * dinv[:, None]).astype(np.float32)
    in_maps = []
    for c in range(NCORES):
        xto = np.zeros((256, RPAD), np.float32)
        xto[:, :RPC] = xd[c * RPC : (c + 1) * RPC].T
        in_maps.append({
            "xtf": xtf,
            "xto": np.ascontiguousarray(xto.reshape(2, 128, RPAD)).astype(ml_dtypes.bfloat16),
            "w1": w1_in, "w2": w2_in, "iota": iota, "ident": ident,
            "dcol1": dcol1[c], "dcol2": dcol2[c],
            "idx": idx16[c], "slots": slots[c],
        })

    trace = bool(int(os.environ.get("GCN_KERNEL_TRACE", "0")))
    try:
        res = run_bass_kernel_spmd(nc, in_maps, core_ids=list(range(NCORES)), trace=trace)
    except Exception:
        # rare transient NRT exec failure: retry once on a fresh dispatch
        time_mod = __import__("time"); time_mod.sleep(2.0)
        res = run_bass_kernel_spmd(nc, in_maps, core_ids=list(range(NCORES)), trace=False)
    kernel.last_results = res
    if trace:
        print(f"HW exec time: {res.exec_time_ns} ns")
        kernel.last_exec_time_ns = res.exec_time_ns

    out = np.concatenate([res.results[c]["out"][:RPC] for c in range(NCORES)], axis=0)
    return out.astype(np.float32)

